# revision 15
# baseline (speedup 1.0000x reference)
"""Trainium2 Bass kernel for EnhancedCondConv2d (moe_routing).

Strategy:
  - data-parallel over batch: 16 samples -> 2 per core on 8 cores
  - routing weights folded into per-sample effective conv weights
    (W_eff[b] = sum_e rw[b,e] * W_e), so the main conv is COUT=256 not E*COUT
  - 3x3 conv = 9 shifted-window f32r matmuls accumulating in PSUM
  - 7x7 spatial attention = channel-contraction matmul -> Q[49, HW] ->
    per-partition shifted DMA copies -> ones-matmul partition reduction
  - BN batch stats via a tiny AllReduce ([128,4] f32) across the 8 cores
"""

import numpy as np

import concourse.bass as bass
import concourse.bacc as bacc
import concourse.tile as tile
from concourse import bass_utils, mybir
from concourse.mybir import AluOpType as alu
from concourse.mybir import ActivationFunctionType as AF

B, CIN, COUT, H, W = 16, 128, 256, 64, 64
E, KK = 4, 3
NCORES = 8
BPC = B // NCORES            # samples per core
HW = H * W                   # 4096
HP, WP = H + 2, W + 2        # 66 (3x3 pad=1)
QH, QW = H + 6, W + 6        # 70 (7x7 pad=3)
NOFF = KK * KK               # 9
NCH = 8                      # spatial chunks per sample
RPC = H // NCH               # rows per chunk
CH = RPC * W                 # 512 elems per chunk
SA_K = 49
f32 = mybir.dt.float32
f32r = mybir.dt.float32r

USE_COLLECTIVE = True

OFFS3 = [(dy, dx) for dy in range(KK) for dx in range(KK)]
OFFS7 = [(dy, dx) for dy in range(7) for dx in range(7)]


# ---------------------------------------------------------------- host prep

def _build_consts(iw):
    """Pack all small weights into one [128, NC] f32 tensor."""
    cols = {}
    buf = np.zeros((128, 1024), np.float32)
    c = 0

    def put(name, arr):
        nonlocal c
        arr = np.asarray(arr, np.float32)
        p, n = arr.shape
        buf[:p, c:c + n] = arr
        cols[name] = c
        c += n

    put('r1wT', iw['r1_w'].T)                       # [128, 2]
    put('ca1a', iw['ca1_w'].T[:128])                # [128, 64]
    put('ca1bw', iw['ca1_w'].T[128:])               # [128, 64]
    wsa = np.asarray(iw['sa_w'], np.float32)[0].reshape(COUT, SA_K)
    put('wsaa', wsa[:128])                          # [128, 49]
    put('wsab', wsa[128:])                          # [128, 49]
    put('gamma', np.asarray(iw['bn_gamma'], np.float32).reshape(2, 128).T)
    put('beta', np.asarray(iw['bn_beta'], np.float32).reshape(2, 128).T)
    put('ca2b', np.asarray(iw['ca2_b'], np.float32).reshape(2, 128).T)
    put('ca2wT', iw['ca2_w'].T)                     # [64, 256]
    put('ones_r', np.ones((1, 128), np.float32))    # bcast stationary
    put('I4', np.eye(4, dtype=np.float32))
    put('r2wT', iw['r2_w'].T)                       # [2, 4]
    put('r3wT', iw['r3_w'].T)                       # [4, 4]
    put('ones49', np.ones((SA_K, 1), np.float32))
    put('r1b', np.asarray(iw['r1_b'])[:, None])
    put('r2b', np.asarray(iw['r2_b'])[:, None])
    put('r3b', np.asarray(iw['r3_b'])[:, None])
    put('ca1b', np.asarray(iw['ca1_b'])[:, None])
    put('sab', np.asarray(iw['sa_b'])[:, None])     # [1, 1]
    put('eps', np.full((128, 1), 1e-5, np.float32))
    nc_ = (c + 31) // 32 * 32
    return np.ascontiguousarray(buf[:, :nc_]), cols


# ---------------------------------------------------------------- kernel IR

def _emit(nc, cols, ncst):
    x_d = nc.dram_tensor("x", [BPC, CIN, H, W], f32, kind="ExternalInput").ap()
    ew_d = nc.dram_tensor("ew", [CIN, E * NOFF * COUT], f32,
                          kind="ExternalInput").ap()
    cst_d = nc.dram_tensor("cst", [128, ncst], f32, kind="ExternalInput").ap()
    y_d = nc.dram_tensor("y", [BPC, COUT, H, W], f32, kind="ExternalOutput").ap()

    with tile.TileContext(nc) as tc:
        _body(tc, x_d, ew_d, cst_d, y_d, cols, ncst)
    nc.compile()
    return nc


def _body(ctx_tc, x_d, ew_d, cst_d, y_d, cols, ncst):
    tc = ctx_tc
    nc = tc.nc

    from contextlib import ExitStack
    ctx = ExitStack()
    with ctx:
        cstp = ctx.enter_context(tc.tile_pool(name="cstp", bufs=1))
        ewp = ctx.enter_context(tc.tile_pool(name="ewp", bufs=2))
        bigp = ctx.enter_context(tc.tile_pool(name="bigp", bufs=3))
        weffp = ctx.enter_context(tc.tile_pool(name="weffp", bufs=2))
        outp = ctx.enter_context(tc.tile_pool(name="outp", bufs=2))
        workp = ctx.enter_context(tc.tile_pool(name="workp", bufs=2))
        sap = ctx.enter_context(tc.tile_pool(name="sap", bufs=2))
        cvps = ctx.enter_context(tc.tile_pool(name="cvps", bufs=3, space="PSUM"))
        qps = ctx.enter_context(tc.tile_pool(name="qps", bufs=1, space="PSUM"))
        sbps = ctx.enter_context(tc.tile_pool(name="sbps", bufs=2, space="PSUM"))
        tinyps = ctx.enter_context(tc.tile_pool(name="tinyps", bufs=2, space="PSUM"))
        dramp = ctx.enter_context(tc.tile_pool(name="dramp", bufs=1, space="DRAM"))

        cst = cstp.tile([128, ncst], f32)
        nc.sync.dma_start(cst, cst_d)

        def C(nm, p, n):
            return cst[0:p, cols[nm]:cols[nm] + n]

        # f32r copies of constants used as f32r matmul operands
        wsar = cstp.tile([128, 2 * SA_K], f32r, tag="wsar")
        wsar = wsar.rearrange("p (h j) -> p h j", h=2)
        nc.scalar.copy(wsar[:, 0, :], C('wsaa', 128, SA_K))
        nc.scalar.copy(wsar[:, 1, :], C('wsab', 128, SA_K))
        ones49r = cstp.tile([SA_K, 1], f32r, tag="ones49r")
        nc.scalar.copy(ones49r, C('ones49', SA_K, 1))
        onesrr = cstp.tile([1, 128], f32r, tag="onesrr")
        nc.scalar.copy(onesrr, C('ones_r', 1, 128))

        # ---------------- load x, build padded x, routing per sample
        xqs, rwbs = [], []
        for b in range(BPC):
            xtmp = bigp.tile([128, HW], f32, tag="big", name=f"xtmp{b}")
            nc.scalar.dma_start(xtmp, x_d[b].opt())
            xq = bigp.tile([128, HP * WP], f32r, tag="big", name=f"xq{b}")
            x3 = xq.rearrange("p (h w) -> p h w", h=HP)
            # zero borders, copy interior (gpsimd; overlaps other work)
            nc.gpsimd.memset(x3[:, 0:1, :].bitcast(f32), 0.0)
            nc.gpsimd.memset(x3[:, HP - 1:HP, :].bitcast(f32), 0.0)
            nc.gpsimd.memset(x3[:, 1:HP - 1, 0:1].bitcast(f32), 0.0)
            nc.gpsimd.memset(x3[:, 1:HP - 1, WP - 1:WP].bitcast(f32), 0.0)
            nc.gpsimd.tensor_copy(
                x3[:, 1:HP - 1, 1:WP - 1],
                xtmp.rearrange("p (h w) -> p h w", h=H))
            xqs.append(xq)

            # routing MLP -> rwb [128, 4] (rw broadcast to all partitions)
            gs = workp.tile([128, 1], f32, tag="gs", name=f"gs{b}")
            nc.vector.reduce_sum(gs, xtmp, axis=mybir.AxisListType.X)
            p1 = tinyps.tile([2, 1], f32, tag="tps", name=f"p1_{b}")
            nc.tensor.matmul(p1, C('r1wT', 128, 2), gs, start=True, stop=True)
            h1 = workp.tile([2, 1], f32, tag="h1", name=f"h1_{b}")
            nc.scalar.activation(h1, p1, AF.Relu, bias=C('r1b', 2, 1),
                                 scale=1.0 / HW)
            p2 = tinyps.tile([4, 1], f32, tag="tps", name=f"p2_{b}")
            nc.tensor.matmul(p2, C('r2wT', 2, 4), h1, start=True, stop=True)
            h2 = workp.tile([4, 1], f32, tag="h2", name=f"h2_{b}")
            nc.scalar.activation(h2, p2, AF.Relu, bias=C('r2b', 4, 1))
            p3 = tinyps.tile([4, 1], f32, tag="tps", name=f"p3_{b}")
            nc.tensor.matmul(p3, C('r3wT', 4, 4), h2, start=True, stop=True)
            e4 = workp.tile([4, 1], f32, tag="e4", name=f"e4_{b}")
            nc.scalar.activation(e4, p3, AF.Exp, bias=C('r3b', 4, 1))
            pse = tinyps.tile([1, 1], f32, tag="tps", name=f"pse{b}")
            nc.tensor.matmul(pse, C('ones49', SA_K, 1)[0:4, :], e4,
                             start=True, stop=True)
            se = workp.tile([1, 1], f32, tag="se", name=f"se{b}")
            nc.scalar.copy(se, pse)
            rcp = workp.tile([1, 1], f32, tag="rcp", name=f"rcp{b}")
            nc.vector.reciprocal(rcp, se)
            psT = tinyps.tile([1, 4], f32, tag="tps", name=f"psT{b}")
            nc.tensor.matmul(psT, e4, C('I4', 4, 4), start=True, stop=True)
            rwT = workp.tile([1, 4], f32, tag="rwT", name=f"rwT{b}")
            nc.vector.tensor_scalar_mul(rwT, psT, rcp)
            psB = tinyps.tile([128, 4], f32, tag="tps", name=f"psB{b}")
            nc.tensor.matmul(psB, C('ones_r', 1, 128), rwT, start=True, stop=True)
            rwb = workp.tile([128, 4], f32, tag="rwb", name=f"rwb{b}")
            nc.scalar.copy(rwb, psB)
            rwbs.append(rwb)

        # ---------------- stream experts, build W_eff per sample
        weffs = [weffp.tile([128, NOFF * COUT], f32r, tag="weff", name=f"weff{b}")
                 for b in range(BPC)]
        for e in range(E):
            ewt = ewp.tile([128, NOFF * COUT], f32, tag="ewt", name=f"ewt{e}")
            nc.sync.dma_start(ewt, ew_d[:, e * NOFF * COUT:(e + 1) * NOFF * COUT])
            for b in range(BPC):
                if e == 0:
                    nc.vector.tensor_scalar_mul(weffs[b], ewt, rwbs[b][:, 0:1])
                else:
                    nc.vector.scalar_tensor_tensor(
                        weffs[b], ewt, rwbs[b][:, e:e + 1], weffs[b],
                        alu.mult, alu.add)

        # ---------------- per-sample state
        obs, ob3s, pcolss = [], [], []
        tsums, sqcolss, cas = [], [], []
        for b in range(BPC):
            ob = outp.tile([128, 2 * HW], f32r, tag="ob", name=f"ob{b}")
            obs.append(ob)
            ob3s.append(ob.rearrange("p (h x) -> p h x", h=2))
            pc = workp.tile([128, 16], f32, tag="pcols", name=f"pcols{b}")
            pcolss.append(pc)
            tsums.append(workp.tile([128, 16], f32, tag="tsum", name=f"tsum{b}"))
            sqcolss.append(workp.tile([128, 16], f32, tag="sqcols",
                                      name=f"sqcols{b}"))
            cas.append(workp.tile([128, 2], f32, tag="ca", name=f"ca{b}"))

        def emit_conv_chunk(b, c):
            x3 = xqs[b].rearrange("p (h w) -> p h w", h=HP)
            for h in range(2):
                ps = cvps.tile([128, CH], f32, tag="cv", name=f"cv{b}_{c}_{h}")
                for o, (dy, dx) in enumerate(OFFS3):
                    lhsT = weffs[b][:, o * COUT + h * 128:
                                    o * COUT + h * 128 + 128]
                    rhs = x3[:, c * RPC + dy:c * RPC + dy + RPC, dx:dx + W]
                    nc.tensor.matmul(ps, lhsT, rhs, start=(o == 0),
                                     stop=(o == NOFF - 1))
                nc.scalar.activation(
                    ob3s[b][:, h, c * CH:(c + 1) * CH], ps, AF.Copy,
                    accum_out=pcolss[b][:, h * NCH + c:h * NCH + c + 1])

        def emit_sa_front(b):
            """Q matmuls, padded-Q assembly, 49 shifted row DMAs."""
            qpad = bigp.tile([SA_K, QH * QW], f32r, tag="big", name=f"qpad{b}")
            q3 = qpad.rearrange("p (r c) -> p r c", r=QH)
            nc.gpsimd.memset(q3[:, 0:3, :].bitcast(f32), 0.0)
            nc.gpsimd.memset(q3[:, QH - 3:QH, :].bitcast(f32), 0.0)
            nc.gpsimd.memset(q3[:, 3:QH - 3, 0:3].bitcast(f32), 0.0)
            nc.gpsimd.memset(q3[:, 3:QH - 3, QW - 3:QW].bitcast(f32), 0.0)
            for c in range(NCH):
                qp = qps.tile([SA_K, CH], f32, tag="q", name=f"q{b}_{c}")
                for h in range(2):
                    nc.tensor.matmul(qp, wsar[:, h, :],
                                     ob3s[b][:, h, c * CH:(c + 1) * CH],
                                     start=(h == 0), stop=(h == 1))
                nc.scalar.copy(
                    q3[:, 3 + c * RPC:3 + (c + 1) * RPC, 3:3 + W],
                    qp.rearrange("p (r c) -> p r c", r=RPC))
            rb = bigp.tile([SA_K, HW], f32r, tag="big", name=f"rb{b}")
            for p, (dy, dx) in enumerate(OFFS7):
                eng = nc.sync if p % 2 == 0 else nc.scalar
                eng.dma_start(rb[p:p + 1, :], q3[p:p + 1, dy:dy + H, dx:dx + W])
            return rb

        def emit_s_chunk(b, rb, c):
            """sa sigmoid, broadcast, t1 = out*sa in place, stats."""
            sp = sbps.tile([1, CH], f32, tag="sb", name=f"sp{b}_{c}")
            nc.tensor.matmul(sp, ones49r, rb[:, c * CH:(c + 1) * CH],
                             start=True, stop=True)
            sac = sap.tile([1, CH], f32r, tag="sas", name=f"sac{b}_{c}")
            nc.scalar.activation(sac, sp, AF.Sigmoid, bias=C('sab', 1, 1))
            bp = sbps.tile([128, CH], f32, tag="sb", name=f"bp{b}_{c}")
            nc.tensor.matmul(bp, onesrr, sac, start=True, stop=True)
            for h in range(2):
                sl = ob3s[b][:, h, c * CH:(c + 1) * CH]
                nc.vector.scalar_tensor_tensor(
                    sl, sl, 1.0, bp, alu.mult, alu.mult,
                    accum_out=tsums[b][:, h * NCH + c:h * NCH + c + 1])
                sqs = workp.tile([128, CH], f32, tag="sqs",
                                 name=f"sqs{b}_{c}_{h}")
                nc.scalar.activation(
                    sqs, sl, AF.Square,
                    accum_out=sqcolss[b][:, h * NCH + c:h * NCH + c + 1])

        def emit_ca(b):
            pg = workp.tile([128, 2], f32, tag="pg", name=f"pg{b}")
            for h in range(2):
                nc.vector.reduce_sum(pg[:, h:h + 1],
                                     pcolss[b][:, h * NCH:(h + 1) * NCH],
                                     axis=mybir.AxisListType.X)
            pca = tinyps.tile([64, 1], f32, tag="tps", name=f"pca{b}")
            for h in range(2):
                nc.tensor.matmul(pca, C('ca1a' if h == 0 else 'ca1bw', 128, 64),
                                 pg[:, h:h + 1], start=(h == 0), stop=(h == 1))
            hca = workp.tile([64, 1], f32, tag="hca", name=f"hca{b}")
            nc.scalar.activation(hca, pca, AF.Relu, bias=C('ca1b', 64, 1),
                                 scale=1.0 / HW)
            for h in range(2):
                pc2 = tinyps.tile([128, 1], f32, tag="tps", name=f"pc2{b}_{h}")
                nc.tensor.matmul(pc2,
                                 C('ca2wT', 64, 256)[:, h * 128:(h + 1) * 128],
                                 hca, start=True, stop=True)
                nc.scalar.activation(cas[b][:, h:h + 1], pc2, AF.Sigmoid,
                                     bias=C('ca2b', 128, 2)[:, h:h + 1])

        # ---------------- main schedule
        # sample0 conv + its Q/R front
        for c in range(NCH):
            emit_conv_chunk(0, c)
        rb0 = emit_sa_front(0)
        emit_ca(0)
        # sample1 conv with sample0's sa/t1/stats interleaved between chunks
        for c in range(NCH):
            emit_conv_chunk(1, c)
            emit_s_chunk(0, rb0, c)
        rb1 = emit_sa_front(1)
        emit_ca(1)
        for c in range(NCH):
            emit_s_chunk(1, rb1, c)

        # ---------------- local stats [128, 4] = (sum h0, sum h1, sq h0, sq h1)
        statL = workp.tile([128, 4], f32, tag="statL")
        tsum2 = [workp.tile([128, 2], f32, tag="tsum2", name=f"tsum2_{b}")
                 for b in range(BPC)]
        sqsum = [workp.tile([128, 2], f32, tag="sqsum", name=f"sqsum{b}")
                 for b in range(BPC)]
        casq = [workp.tile([128, 2], f32, tag="casq", name=f"casq{b}")
                for b in range(BPC)]
        tmp2 = [workp.tile([128, 2], f32, tag="tmp2", name=f"tmp2_{b}")
                for b in range(BPC)]
        for b in range(BPC):
            for h in range(2):
                nc.vector.reduce_sum(tsum2[b][:, h:h + 1],
                                     tsums[b][:, h * NCH:(h + 1) * NCH],
                                     axis=mybir.AxisListType.X)
                nc.vector.reduce_sum(sqsum[b][:, h:h + 1],
                                     sqcolss[b][:, h * NCH:(h + 1) * NCH],
                                     axis=mybir.AxisListType.X)
            nc.vector.tensor_mul(casq[b], cas[b], cas[b])
            if b == 0:
                nc.vector.tensor_mul(statL[:, 0:2], cas[b], tsum2[b])
                nc.vector.tensor_mul(statL[:, 2:4], casq[b], sqsum[b])
            else:
                nc.vector.tensor_mul(tmp2[b], cas[b], tsum2[b])
                nc.vector.tensor_add(statL[:, 0:2], statL[:, 0:2], tmp2[b])
                nc.vector.tensor_mul(tmp2[b], casq[b], sqsum[b])
                nc.vector.tensor_add(statL[:, 2:4], statL[:, 2:4], tmp2[b])

        # ---------------- AllReduce across 8 cores
        statG = workp.tile([128, 4], f32, tag="statG")
        if USE_COLLECTIVE:
            cin_t = dramp.tile([128, 4], f32, tag="cin")
            cout_t = dramp.tile([128, 4], f32, tag="cout")
            nc.sync.dma_start(cin_t, statL)
            nc.gpsimd.collective_compute(
                "AllReduce", alu.add,
                replica_groups=[list(range(NCORES))],
                ins=[cin_t.opt()], outs=[cout_t.opt()])
            nc.sync.dma_start(statG, cout_t)
        else:
            # bisection mode: local stats scaled by NCORES (approximate BN)
            nc.vector.tensor_scalar_mul(statG, statL, float(NCORES))

        # ---------------- finalize: y = relu(t1 * (ca*s) + (beta - mean*s))
        NTOT = float(B * HW)
        mean = workp.tile([128, 2], f32, tag="mean")
        nc.vector.tensor_scalar_mul(mean, statG[:, 0:2], 1.0 / NTOT)
        msq = workp.tile([128, 2], f32, tag="msq")
        nc.vector.tensor_mul(msq, mean, mean)
        var = workp.tile([128, 2], f32, tag="var")
        nc.vector.scalar_tensor_tensor(var, statG[:, 2:4], 1.0 / NTOT, msq,
                                       alu.mult, alu.subtract)
        stdv = workp.tile([128, 2], f32, tag="stdv")
        nc.scalar.activation(stdv, var, AF.Sqrt, bias=C('eps', 128, 1))
        rstd = workp.tile([128, 2], f32, tag="rstd")
        nc.vector.reciprocal(rstd, stdv)
        sg = workp.tile([128, 2], f32, tag="sg")
        nc.vector.tensor_mul(sg, C('gamma', 128, 2), rstd)
        ms = workp.tile([128, 2], f32, tag="ms")
        nc.vector.tensor_mul(ms, mean, sg)
        tg = workp.tile([128, 2], f32, tag="tg")
        nc.vector.tensor_sub(tg, C('beta', 128, 2), ms)

        for b in range(BPC):
            casg = workp.tile([128, 2], f32, tag="casg", name=f"casg{b}")
            nc.vector.tensor_mul(casg, cas[b], sg)
            for h in range(2):
                nc.scalar.activation(ob3s[b][:, h, :], ob3s[b][:, h, :],
                                     AF.Relu, scale=casg[:, h:h + 1],
                                     bias=tg[:, h:h + 1])
                nc.sync.dma_start(y_d[b, h * 128:(h + 1) * 128].opt(),
                                  ob3s[b][:, h, :].bitcast(f32))


# ---------------------------------------------------------------- driver

_CACHE = {}


def _get_nc(ncst, cols_key, cols):
    key = (ncst, cols_key)
    if key not in _CACHE:
        nc = bacc.Bacc("TRN2", target_bir_lowering=False, debug=False,
                       num_devices=NCORES)
        _CACHE[key] = _emit(nc, cols, ncst)
    return _CACHE[key]


def _run(inputs, trace=False):
    cst, cols = _build_consts(inputs)
    ncst = cst.shape[1]
    nc = _get_nc(ncst, tuple(sorted(cols.items())), cols)

    x = np.ascontiguousarray(np.asarray(inputs['x'], np.float32))
    ew = np.ascontiguousarray(
        np.asarray(inputs['expert_w'], np.float32)
        .transpose(2, 0, 3, 4, 1).reshape(CIN, E * NOFF * COUT))

    in_maps = []
    for c in range(NCORES):
        in_maps.append({
            "x": np.ascontiguousarray(x[c * BPC:(c + 1) * BPC]),
            "ew": ew,
            "cst": cst,
        })
    br = bass_utils.run_bass_kernel_spmd(
        nc, in_maps, core_ids=list(range(NCORES)), trace=trace)
    y = np.concatenate([r["y"] for r in br.results], axis=0)
    return y.astype(np.float32, copy=False), br


def kernel(**inputs):
    y, _ = _run(inputs, trace=False)
    return y


# revision 16
# speedup vs baseline: 2.7282x; 2.7282x over previous
"""Trainium2 Bass kernel for EnhancedCondConv2d (moe_routing).

Strategy:
  - data-parallel over batch: 16 samples -> 2 per core on 8 cores
  - routing weights folded into per-sample effective conv weights
    (W_eff[b] = sum_e rw[b,e] * W_e), so the main conv is COUT=256 not E*COUT
  - 3x3 conv = 9 shifted-window f32r matmuls accumulating in PSUM
  - 7x7 spatial attention = channel-contraction matmul -> Q[49, HW] ->
    per-partition shifted DMA copies -> ones-matmul partition reduction
  - BN batch stats via a tiny AllReduce ([128,4] f32) across the 8 cores
"""

import numpy as np

import concourse.bass as bass
import concourse.bacc as bacc
import concourse.tile as tile
from concourse import bass_utils, mybir
from concourse.mybir import AluOpType as alu
from concourse.mybir import ActivationFunctionType as AF

B, CIN, COUT, H, W = 16, 128, 256, 64, 64
E, KK = 4, 3
NCORES = 8
BPC = B // NCORES            # samples per core
HW = H * W                   # 4096
HP, WP = H + 2, W + 2        # 66 (3x3 pad=1)
QH, QW = H + 6, W + 6        # 70 (7x7 pad=3)
NOFF = KK * KK               # 9
NCH = 8                      # spatial chunks per sample
RPC = H // NCH               # rows per chunk
CH = RPC * W                 # 512 elems per chunk
SA_K = 49
f32 = mybir.dt.float32
f32r = mybir.dt.float32r

USE_COLLECTIVE = True

OFFS3 = [(dy, dx) for dy in range(KK) for dx in range(KK)]
OFFS7 = [(dy, dx) for dy in range(7) for dx in range(7)]


# ---------------------------------------------------------------- host prep

def _build_consts(iw):
    """Pack all small weights into one [128, NC] f32 tensor."""
    cols = {}
    buf = np.zeros((128, 1024), np.float32)
    c = 0

    def put(name, arr):
        nonlocal c
        arr = np.asarray(arr, np.float32)
        p, n = arr.shape
        buf[:p, c:c + n] = arr
        cols[name] = c
        c += n

    put('r1wT', iw['r1_w'].T)                       # [128, 2]
    put('ca1a', iw['ca1_w'].T[:128])                # [128, 64]
    put('ca1bw', iw['ca1_w'].T[128:])               # [128, 64]
    wsa = np.asarray(iw['sa_w'], np.float32)[0].reshape(COUT, SA_K)
    put('wsaa', wsa[:128])                          # [128, 49]
    put('wsab', wsa[128:])                          # [128, 49]
    put('gamma', np.asarray(iw['bn_gamma'], np.float32).reshape(2, 128).T)
    put('beta', np.asarray(iw['bn_beta'], np.float32).reshape(2, 128).T)
    put('ca2b', np.asarray(iw['ca2_b'], np.float32).reshape(2, 128).T)
    put('ca2wT', iw['ca2_w'].T)                     # [64, 256]
    put('ones_r', np.ones((1, 128), np.float32))    # bcast stationary
    put('I4', np.eye(4, dtype=np.float32))
    put('r2wT', iw['r2_w'].T)                       # [2, 4]
    put('r3wT', iw['r3_w'].T)                       # [4, 4]
    put('ones49', np.ones((SA_K, 1), np.float32))
    put('r1b', np.asarray(iw['r1_b'])[:, None])
    put('r2b', np.asarray(iw['r2_b'])[:, None])
    put('r3b', np.asarray(iw['r3_b'])[:, None])
    put('ca1b', np.asarray(iw['ca1_b'])[:, None])
    put('sab', np.asarray(iw['sa_b'])[:, None])     # [1, 1]
    put('eps', np.full((128, 1), 1e-5, np.float32))
    nc_ = (c + 31) // 32 * 32
    return np.ascontiguousarray(buf[:, :nc_]), cols


# ---------------------------------------------------------------- kernel IR

def _emit(nc, cols, ncst):
    x_d = nc.dram_tensor("x", [BPC, CIN, H, W], f32, kind="ExternalInput").ap()
    ew_d = nc.dram_tensor("ew", [CIN, E * NOFF * COUT], f32,
                          kind="ExternalInput").ap()
    cst_d = nc.dram_tensor("cst", [128, ncst], f32, kind="ExternalInput").ap()
    y_d = nc.dram_tensor("y", [BPC, COUT, H, W], f32, kind="ExternalOutput").ap()

    with tile.TileContext(nc) as tc:
        _body(tc, x_d, ew_d, cst_d, y_d, cols, ncst)
    nc.compile()
    return nc


def _body(ctx_tc, x_d, ew_d, cst_d, y_d, cols, ncst):
    tc = ctx_tc
    nc = tc.nc

    from contextlib import ExitStack
    ctx = ExitStack()
    with ctx:
        cstp = ctx.enter_context(tc.tile_pool(name="cstp", bufs=1))
        ewp = ctx.enter_context(tc.tile_pool(name="ewp", bufs=2))
        bigp = ctx.enter_context(tc.tile_pool(name="bigp", bufs=3))
        weffp = ctx.enter_context(tc.tile_pool(name="weffp", bufs=2))
        outp = ctx.enter_context(tc.tile_pool(name="outp", bufs=2))
        workp = ctx.enter_context(tc.tile_pool(name="workp", bufs=2))
        sap = ctx.enter_context(tc.tile_pool(name="sap", bufs=2))
        cvps = ctx.enter_context(tc.tile_pool(name="cvps", bufs=3, space="PSUM"))
        qps = ctx.enter_context(tc.tile_pool(name="qps", bufs=1, space="PSUM"))
        sbps = ctx.enter_context(tc.tile_pool(name="sbps", bufs=2, space="PSUM"))
        tinyps = ctx.enter_context(tc.tile_pool(name="tinyps", bufs=2, space="PSUM"))
        dramp = ctx.enter_context(tc.tile_pool(name="dramp", bufs=1, space="DRAM"))

        cst = cstp.tile([128, ncst], f32)
        nc.sync.dma_start(cst, cst_d)

        def C(nm, p, n):
            return cst[0:p, cols[nm]:cols[nm] + n]

        # f32r copies of constants used as f32r matmul operands
        wsar = cstp.tile([128, 2 * SA_K], f32r, tag="wsar")
        wsar = wsar.rearrange("p (h j) -> p h j", h=2)
        nc.scalar.copy(wsar[:, 0, :], C('wsaa', 128, SA_K))
        nc.scalar.copy(wsar[:, 1, :], C('wsab', 128, SA_K))
        ones49r = cstp.tile([SA_K, 1], f32r, tag="ones49r")
        nc.scalar.copy(ones49r, C('ones49', SA_K, 1))
        onesrr = cstp.tile([1, 128], f32r, tag="onesrr")
        nc.scalar.copy(onesrr, C('ones_r', 1, 128))

        # ---------------- load x, build padded x, routing per sample
        xqs, rwbs = [], []
        for b in range(BPC):
            xtmp = bigp.tile([128, HW], f32, tag="big", name=f"xtmp{b}")
            nc.scalar.dma_start(xtmp, x_d[b].opt())
            xq = bigp.tile([128, HP * WP], f32r, tag="big", name=f"xq{b}")
            x3 = xq.rearrange("p (h w) -> p h w", h=HP)
            # zero borders, copy interior (gpsimd; overlaps other work)
            nc.gpsimd.memset(x3[:, 0:1, :].bitcast(f32), 0.0)
            nc.gpsimd.memset(x3[:, HP - 1:HP, :].bitcast(f32), 0.0)
            nc.gpsimd.memset(x3[:, 1:HP - 1, 0:1].bitcast(f32), 0.0)
            nc.gpsimd.memset(x3[:, 1:HP - 1, WP - 1:WP].bitcast(f32), 0.0)
            nc.gpsimd.tensor_copy(
                x3[:, 1:HP - 1, 1:WP - 1],
                xtmp.rearrange("p (h w) -> p h w", h=H))
            xqs.append(xq)

            # routing MLP -> rwb [128, 4] (rw broadcast to all partitions)
            gs = workp.tile([128, 1], f32, tag="gs", name=f"gs{b}")
            nc.vector.reduce_sum(gs, xtmp, axis=mybir.AxisListType.X)
            p1 = tinyps.tile([2, 1], f32, tag="tps", name=f"p1_{b}")
            nc.tensor.matmul(p1, C('r1wT', 128, 2), gs, start=True, stop=True)
            h1 = workp.tile([2, 1], f32, tag="h1", name=f"h1_{b}")
            nc.scalar.activation(h1, p1, AF.Relu, bias=C('r1b', 2, 1),
                                 scale=1.0 / HW)
            p2 = tinyps.tile([4, 1], f32, tag="tps", name=f"p2_{b}")
            nc.tensor.matmul(p2, C('r2wT', 2, 4), h1, start=True, stop=True)
            h2 = workp.tile([4, 1], f32, tag="h2", name=f"h2_{b}")
            nc.scalar.activation(h2, p2, AF.Relu, bias=C('r2b', 4, 1))
            p3 = tinyps.tile([4, 1], f32, tag="tps", name=f"p3_{b}")
            nc.tensor.matmul(p3, C('r3wT', 4, 4), h2, start=True, stop=True)
            e4 = workp.tile([4, 1], f32, tag="e4", name=f"e4_{b}")
            nc.scalar.activation(e4, p3, AF.Exp, bias=C('r3b', 4, 1))
            pse = tinyps.tile([1, 1], f32, tag="tps", name=f"pse{b}")
            nc.tensor.matmul(pse, C('ones49', SA_K, 1)[0:4, :], e4,
                             start=True, stop=True)
            se = workp.tile([1, 1], f32, tag="se", name=f"se{b}")
            nc.scalar.copy(se, pse)
            rcp = workp.tile([1, 1], f32, tag="rcp", name=f"rcp{b}")
            nc.vector.reciprocal(rcp, se)
            psT = tinyps.tile([1, 4], f32, tag="tps", name=f"psT{b}")
            nc.tensor.matmul(psT, e4, C('I4', 4, 4), start=True, stop=True)
            rwT = workp.tile([1, 4], f32, tag="rwT", name=f"rwT{b}")
            nc.vector.tensor_scalar_mul(rwT, psT, rcp)
            psB = tinyps.tile([128, 4], f32, tag="tps", name=f"psB{b}")
            nc.tensor.matmul(psB, C('ones_r', 1, 128), rwT, start=True, stop=True)
            rwb = workp.tile([128, 4], f32, tag="rwb", name=f"rwb{b}")
            nc.scalar.copy(rwb, psB)
            rwbs.append(rwb)

        # ---------------- stream experts, build W_eff per sample
        weffs = [weffp.tile([128, NOFF * COUT], f32r, tag="weff", name=f"weff{b}")
                 for b in range(BPC)]
        for e in range(E):
            ewt = ewp.tile([128, NOFF * COUT], f32, tag="ewt", name=f"ewt{e}")
            nc.sync.dma_start(ewt, ew_d[:, e * NOFF * COUT:(e + 1) * NOFF * COUT])
            for b in range(BPC):
                if e == 0:
                    nc.vector.tensor_scalar_mul(weffs[b], ewt, rwbs[b][:, 0:1])
                else:
                    nc.vector.scalar_tensor_tensor(
                        weffs[b], ewt, rwbs[b][:, e:e + 1], weffs[b],
                        alu.mult, alu.add)

        # ---------------- per-sample state
        obs, ob3s, pcolss = [], [], []
        tsums, sqcolss, cas = [], [], []
        for b in range(BPC):
            ob = outp.tile([128, 2 * HW], f32r, tag="ob", name=f"ob{b}")
            obs.append(ob)
            ob3s.append(ob.rearrange("p (h x) -> p h x", h=2))
            pc = workp.tile([128, 16], f32, tag="pcols", name=f"pcols{b}")
            pcolss.append(pc)
            tsums.append(workp.tile([128, 16], f32, tag="tsum", name=f"tsum{b}"))
            sqcolss.append(workp.tile([128, 16], f32, tag="sqcols",
                                      name=f"sqcols{b}"))
            cas.append(workp.tile([128, 2], f32, tag="ca", name=f"ca{b}"))

        def emit_conv_chunk(b, c):
            x3 = xqs[b].rearrange("p (h w) -> p h w", h=HP)
            for h in range(2):
                ps = cvps.tile([128, CH], f32, tag="cv", name=f"cv{b}_{c}_{h}")
                for o, (dy, dx) in enumerate(OFFS3):
                    lhsT = weffs[b][:, o * COUT + h * 128:
                                    o * COUT + h * 128 + 128]
                    rhs = x3[:, c * RPC + dy:c * RPC + dy + RPC, dx:dx + W]
                    nc.tensor.matmul(ps, lhsT, rhs, start=(o == 0),
                                     stop=(o == NOFF - 1))
                nc.scalar.activation(
                    ob3s[b][:, h, c * CH:(c + 1) * CH], ps, AF.Copy,
                    accum_out=pcolss[b][:, h * NCH + c:h * NCH + c + 1])

        def emit_sa_front(b):
            """Q matmuls, padded-Q assembly, 49 shifted row DMAs."""
            qpad = bigp.tile([SA_K, QH * QW], f32r, tag="big", name=f"qpad{b}")
            q3 = qpad.rearrange("p (r c) -> p r c", r=QH)
            nc.gpsimd.memset(q3[:, 0:3, :].bitcast(f32), 0.0)
            nc.gpsimd.memset(q3[:, QH - 3:QH, :].bitcast(f32), 0.0)
            nc.gpsimd.memset(q3[:, 3:QH - 3, 0:3].bitcast(f32), 0.0)
            nc.gpsimd.memset(q3[:, 3:QH - 3, QW - 3:QW].bitcast(f32), 0.0)
            for c in range(NCH):
                qp = qps.tile([SA_K, CH], f32, tag="q", name=f"q{b}_{c}")
                for h in range(2):
                    nc.tensor.matmul(qp, wsar[:, h, :],
                                     ob3s[b][:, h, c * CH:(c + 1) * CH],
                                     start=(h == 0), stop=(h == 1))
                nc.scalar.copy(
                    q3[:, 3 + c * RPC:3 + (c + 1) * RPC, 3:3 + W],
                    qp.rearrange("p (r c) -> p r c", r=RPC))
            rb = bigp.tile([SA_K, HW], f32r, tag="big", name=f"rb{b}")
            for p, (dy, dx) in enumerate(OFFS7):
                eng = nc.sync if p % 2 == 0 else nc.scalar
                eng.dma_start(rb[p:p + 1, :], q3[p:p + 1, dy:dy + H, dx:dx + W])
            return rb

        def emit_s_chunk(b, rb, c):
            """sa sigmoid, broadcast, t1 = out*sa in place, stats."""
            sp = sbps.tile([1, CH], f32, tag="sb", name=f"sp{b}_{c}")
            nc.tensor.matmul(sp, ones49r, rb[:, c * CH:(c + 1) * CH],
                             start=True, stop=True)
            sac = sap.tile([1, CH], f32r, tag="sas", name=f"sac{b}_{c}")
            nc.scalar.activation(sac, sp, AF.Sigmoid, bias=C('sab', 1, 1))
            bp = sbps.tile([128, CH], f32, tag="sb", name=f"bp{b}_{c}")
            nc.tensor.matmul(bp, onesrr, sac, start=True, stop=True)
            for h in range(2):
                sl = ob3s[b][:, h, c * CH:(c + 1) * CH]
                nc.vector.scalar_tensor_tensor(
                    sl, sl, 1.0, bp, alu.mult, alu.mult,
                    accum_out=tsums[b][:, h * NCH + c:h * NCH + c + 1])
                sqs = workp.tile([128, CH], f32, tag="sqs",
                                 name=f"sqs{b}_{c}_{h}")
                nc.scalar.activation(
                    sqs, sl, AF.Square,
                    accum_out=sqcolss[b][:, h * NCH + c:h * NCH + c + 1])

        def emit_ca(b):
            pg = workp.tile([128, 2], f32, tag="pg", name=f"pg{b}")
            for h in range(2):
                nc.vector.reduce_sum(pg[:, h:h + 1],
                                     pcolss[b][:, h * NCH:(h + 1) * NCH],
                                     axis=mybir.AxisListType.X)
            pca = tinyps.tile([64, 1], f32, tag="tps", name=f"pca{b}")
            for h in range(2):
                nc.tensor.matmul(pca, C('ca1a' if h == 0 else 'ca1bw', 128, 64),
                                 pg[:, h:h + 1], start=(h == 0), stop=(h == 1))
            hca = workp.tile([64, 1], f32, tag="hca", name=f"hca{b}")
            nc.scalar.activation(hca, pca, AF.Relu, bias=C('ca1b', 64, 1),
                                 scale=1.0 / HW)
            for h in range(2):
                pc2 = tinyps.tile([128, 1], f32, tag="tps", name=f"pc2{b}_{h}")
                nc.tensor.matmul(pc2,
                                 C('ca2wT', 64, 256)[:, h * 128:(h + 1) * 128],
                                 hca, start=True, stop=True)
                nc.scalar.activation(cas[b][:, h:h + 1], pc2, AF.Sigmoid,
                                     bias=C('ca2b', 128, 2)[:, h:h + 1])

        # ---------------- main schedule
        # sample0 conv + its Q/R front
        for c in range(NCH):
            emit_conv_chunk(0, c)
        rb0 = emit_sa_front(0)
        emit_ca(0)
        # sample1 conv with sample0's sa/t1/stats interleaved between chunks
        for c in range(NCH):
            emit_conv_chunk(1, c)
            emit_s_chunk(0, rb0, c)
        rb1 = emit_sa_front(1)
        emit_ca(1)
        for c in range(NCH):
            emit_s_chunk(1, rb1, c)

        # ---------------- local stats [128, 4] = (sum h0, sum h1, sq h0, sq h1)
        statL = workp.tile([128, 4], f32, tag="statL")
        tsum2 = [workp.tile([128, 2], f32, tag="tsum2", name=f"tsum2_{b}")
                 for b in range(BPC)]
        sqsum = [workp.tile([128, 2], f32, tag="sqsum", name=f"sqsum{b}")
                 for b in range(BPC)]
        casq = [workp.tile([128, 2], f32, tag="casq", name=f"casq{b}")
                for b in range(BPC)]
        tmp2 = [workp.tile([128, 2], f32, tag="tmp2", name=f"tmp2_{b}")
                for b in range(BPC)]
        for b in range(BPC):
            for h in range(2):
                nc.vector.reduce_sum(tsum2[b][:, h:h + 1],
                                     tsums[b][:, h * NCH:(h + 1) * NCH],
                                     axis=mybir.AxisListType.X)
                nc.vector.reduce_sum(sqsum[b][:, h:h + 1],
                                     sqcolss[b][:, h * NCH:(h + 1) * NCH],
                                     axis=mybir.AxisListType.X)
            nc.vector.tensor_mul(casq[b], cas[b], cas[b])
            if b == 0:
                nc.vector.tensor_mul(statL[:, 0:2], cas[b], tsum2[b])
                nc.vector.tensor_mul(statL[:, 2:4], casq[b], sqsum[b])
            else:
                nc.vector.tensor_mul(tmp2[b], cas[b], tsum2[b])
                nc.vector.tensor_add(statL[:, 0:2], statL[:, 0:2], tmp2[b])
                nc.vector.tensor_mul(tmp2[b], casq[b], sqsum[b])
                nc.vector.tensor_add(statL[:, 2:4], statL[:, 2:4], tmp2[b])

        # ---------------- AllReduce across 8 cores
        statG = workp.tile([128, 4], f32, tag="statG")
        if USE_COLLECTIVE:
            cin_t = dramp.tile([128, 4], f32, tag="cin")
            cout_t = dramp.tile([128, 4], f32, tag="cout")
            nc.sync.dma_start(cin_t, statL)
            nc.gpsimd.collective_compute(
                "AllReduce", alu.add,
                replica_groups=[list(range(NCORES))],
                ins=[cin_t.opt()], outs=[cout_t.opt()])
            nc.sync.dma_start(statG, cout_t)
        else:
            # bisection mode: local stats scaled by NCORES (approximate BN)
            nc.vector.tensor_scalar_mul(statG, statL, float(NCORES))

        # ---------------- finalize: y = relu(t1 * (ca*s) + (beta - mean*s))
        NTOT = float(B * HW)
        mean = workp.tile([128, 2], f32, tag="mean")
        nc.vector.tensor_scalar_mul(mean, statG[:, 0:2], 1.0 / NTOT)
        msq = workp.tile([128, 2], f32, tag="msq")
        nc.vector.tensor_mul(msq, mean, mean)
        var = workp.tile([128, 2], f32, tag="var")
        nc.vector.scalar_tensor_tensor(var, statG[:, 2:4], 1.0 / NTOT, msq,
                                       alu.mult, alu.subtract)
        stdv = workp.tile([128, 2], f32, tag="stdv")
        nc.scalar.activation(stdv, var, AF.Sqrt, bias=C('eps', 128, 1))
        rstd = workp.tile([128, 2], f32, tag="rstd")
        nc.vector.reciprocal(rstd, stdv)
        sg = workp.tile([128, 2], f32, tag="sg")
        nc.vector.tensor_mul(sg, C('gamma', 128, 2), rstd)
        ms = workp.tile([128, 2], f32, tag="ms")
        nc.vector.tensor_mul(ms, mean, sg)
        tg = workp.tile([128, 2], f32, tag="tg")
        nc.vector.tensor_sub(tg, C('beta', 128, 2), ms)

        for b in range(BPC):
            casg = workp.tile([128, 2], f32, tag="casg", name=f"casg{b}")
            nc.vector.tensor_mul(casg, cas[b], sg)
            for h in range(2):
                nc.scalar.activation(ob3s[b][:, h, :], ob3s[b][:, h, :],
                                     AF.Relu, scale=casg[:, h:h + 1],
                                     bias=tg[:, h:h + 1])
                nc.sync.dma_start(y_d[b, h * 128:(h + 1) * 128].opt(),
                                  ob3s[b][:, h, :].bitcast(f32))


# ---------------------------------------------------------------- driver

_CACHE = {}


def _get_nc(ncst, cols_key, cols):
    key = (ncst, cols_key, USE_COLLECTIVE)
    if key not in _CACHE:
        nc = bacc.Bacc("TRN2", target_bir_lowering=False, debug=False,
                       num_devices=NCORES)
        _CACHE[key] = _emit(nc, cols, ncst)
    return _CACHE[key]


def _run(inputs, trace=False):
    cst, cols = _build_consts(inputs)
    ncst = cst.shape[1]
    nc = _get_nc(ncst, tuple(sorted(cols.items())), cols)

    x = np.ascontiguousarray(np.asarray(inputs['x'], np.float32))
    ew = np.ascontiguousarray(
        np.asarray(inputs['expert_w'], np.float32)
        .transpose(2, 0, 3, 4, 1).reshape(CIN, E * NOFF * COUT))

    in_maps = []
    for c in range(NCORES):
        in_maps.append({
            "x": np.ascontiguousarray(x[c * BPC:(c + 1) * BPC]),
            "ew": ew,
            "cst": cst,
        })
    br = bass_utils.run_bass_kernel_spmd(
        nc, in_maps, core_ids=list(range(NCORES)), trace=trace)
    y = np.concatenate([r["y"] for r in br.results], axis=0)
    return y.astype(np.float32, copy=False), br


def kernel(**inputs):
    y, _ = _run(inputs, trace=False)
    return y


# revision 18
# speedup vs baseline: 366.3158x; 134.2690x over previous
"""Trainium2 Bass kernel for EnhancedCondConv2d (moe_routing).

Strategy:
  - data-parallel over batch: 16 samples -> 2 per core on 8 cores
  - routing weights folded into per-sample effective conv weights
    (W_eff[b] = sum_e rw[b,e] * W_e), so the main conv is COUT=256 not E*COUT
  - 3x3 conv = 9 shifted-window f32r matmuls accumulating in PSUM
  - 7x7 spatial attention = channel-contraction matmul -> Q[49, HW] ->
    per-partition shifted DMA copies -> ones-matmul partition reduction
  - BN batch stats via a tiny AllReduce ([128,4] f32) across the 8 cores
"""

import numpy as np

import concourse.bass as bass
import concourse.bacc as bacc
import concourse.tile as tile
from concourse import bass_utils, mybir
from concourse.mybir import AluOpType as alu
from concourse.mybir import ActivationFunctionType as AF

B, CIN, COUT, H, W = 16, 128, 256, 64, 64
E, KK = 4, 3
NCORES = 8
BPC = B // NCORES            # samples per core
HW = H * W                   # 4096
HP, WP = H + 2, W + 2        # 66 (3x3 pad=1)
QH, QW = H + 6, W + 6        # 70 (7x7 pad=3)
NOFF = KK * KK               # 9
NCH = 8                      # spatial chunks per sample
RPC = H // NCH               # rows per chunk
CH = RPC * W                 # 512 elems per chunk
SA_K = 49
RSP = 70 * 64                   # 4480: shifted-span length for sa reduction
QPS = QH * QW + 6               # qpad + tail so span reads stay in bounds
f32 = mybir.dt.float32
f32r = mybir.dt.float32r

USE_COLLECTIVE = True

OFFS3 = [(dy, dx) for dy in range(KK) for dx in range(KK)]
OFFS7 = [(dy, dx) for dy in range(7) for dx in range(7)]


# ---------------------------------------------------------------- host prep

def _build_consts(iw):
    """Pack all small weights into one [128, NC] f32 tensor."""
    cols = {}
    buf = np.zeros((128, 1024), np.float32)
    c = 0

    def put(name, arr):
        nonlocal c
        arr = np.asarray(arr, np.float32)
        p, n = arr.shape
        buf[:p, c:c + n] = arr
        cols[name] = c
        c += n

    put('r1wT', iw['r1_w'].T)                       # [128, 2]
    put('ca1a', iw['ca1_w'].T[:128])                # [128, 64]
    put('ca1bw', iw['ca1_w'].T[128:])               # [128, 64]
    wsa = np.asarray(iw['sa_w'], np.float32)[0].reshape(COUT, SA_K)
    put('wsaa', wsa[:128])                          # [128, 49]
    put('wsab', wsa[128:])                          # [128, 49]
    put('gamma', np.asarray(iw['bn_gamma'], np.float32).reshape(2, 128).T)
    put('beta', np.asarray(iw['bn_beta'], np.float32).reshape(2, 128).T)
    put('ca2b', np.asarray(iw['ca2_b'], np.float32).reshape(2, 128).T)
    put('ca2wT', iw['ca2_w'].T)                     # [64, 256]
    put('ones_r', np.ones((1, 128), np.float32))    # bcast stationary
    put('I4', np.eye(4, dtype=np.float32))
    put('r2wT', iw['r2_w'].T)                       # [2, 4]
    put('r3wT', iw['r3_w'].T)                       # [4, 4]
    put('ones49', np.ones((SA_K, 1), np.float32))
    put('r1b', np.asarray(iw['r1_b'])[:, None])
    put('r2b', np.asarray(iw['r2_b'])[:, None])
    put('r3b', np.asarray(iw['r3_b'])[:, None])
    put('ca1b', np.asarray(iw['ca1_b'])[:, None])
    put('sab', np.asarray(iw['sa_b'])[:, None])     # [1, 1]
    put('eps', np.full((128, 1), 1e-5, np.float32))
    nc_ = (c + 31) // 32 * 32
    return np.ascontiguousarray(buf[:, :nc_]), cols


# ---------------------------------------------------------------- kernel IR

def _emit(nc, cols, ncst):
    x_d = nc.dram_tensor("x", [BPC, CIN, H, W], f32, kind="ExternalInput").ap()
    ew_d = nc.dram_tensor("ew", [CIN, E * NOFF * COUT], f32,
                          kind="ExternalInput").ap()
    cst_d = nc.dram_tensor("cst", [128, ncst], f32, kind="ExternalInput").ap()
    y_d = nc.dram_tensor("y", [BPC, COUT, H, W], f32, kind="ExternalOutput").ap()

    with tile.TileContext(nc) as tc:
        _body(tc, x_d, ew_d, cst_d, y_d, cols, ncst)
    nc.compile()
    return nc


def _body(ctx_tc, x_d, ew_d, cst_d, y_d, cols, ncst):
    tc = ctx_tc
    nc = tc.nc

    from contextlib import ExitStack
    ctx = ExitStack()
    with ctx:
        cstp = ctx.enter_context(tc.tile_pool(name="cstp", bufs=1))
        ewp = ctx.enter_context(tc.tile_pool(name="ewp", bufs=2))
        bigp = ctx.enter_context(tc.tile_pool(name="bigp", bufs=3))
        weffp = ctx.enter_context(tc.tile_pool(name="weffp", bufs=2))
        outp = ctx.enter_context(tc.tile_pool(name="outp", bufs=2))
        workp = ctx.enter_context(tc.tile_pool(name="workp", bufs=2))
        sap = ctx.enter_context(tc.tile_pool(name="sap", bufs=2))
        cvps = ctx.enter_context(tc.tile_pool(name="cvps", bufs=3, space="PSUM"))
        qps = ctx.enter_context(tc.tile_pool(name="qps", bufs=1, space="PSUM"))
        sbps = ctx.enter_context(tc.tile_pool(name="sbps", bufs=2, space="PSUM"))
        tinyps = ctx.enter_context(tc.tile_pool(name="tinyps", bufs=2, space="PSUM"))
        dramp = ctx.enter_context(tc.tile_pool(name="dramp", bufs=1, space="DRAM"))

        cst = cstp.tile([128, ncst], f32)
        nc.sync.dma_start(cst, cst_d)

        def C(nm, p, n):
            return cst[0:p, cols[nm]:cols[nm] + n]

        # f32r copies of constants used as f32r matmul operands
        wsar = cstp.tile([128, 2 * SA_K], f32r, tag="wsar")
        wsar = wsar.rearrange("p (h j) -> p h j", h=2)
        nc.scalar.copy(wsar[:, 0, :], C('wsaa', 128, SA_K))
        nc.scalar.copy(wsar[:, 1, :], C('wsab', 128, SA_K))
        ones49r = cstp.tile([SA_K, 1], f32r, tag="ones49r")
        nc.scalar.copy(ones49r, C('ones49', SA_K, 1))
        onesrr = cstp.tile([1, 128], f32r, tag="onesrr")
        nc.scalar.copy(onesrr, C('ones_r', 1, 128))

        # ---------------- load x, build padded x, routing per sample
        xqs, rwbs = [], []
        for b in range(BPC):
            xtmp = bigp.tile([128, HW], f32, tag="big", name=f"xtmp{b}")
            nc.scalar.dma_start(xtmp, x_d[b].opt())
            xq = bigp.tile([128, HP * WP], f32r, tag="big", name=f"xq{b}")
            x3 = xq.rearrange("p (h w) -> p h w", h=HP)
            # zero borders, copy interior (gpsimd; overlaps other work)
            nc.gpsimd.memset(x3[:, 0:1, :].bitcast(f32), 0.0)
            nc.gpsimd.memset(x3[:, HP - 1:HP, :].bitcast(f32), 0.0)
            nc.gpsimd.memset(x3[:, 1:HP - 1, 0:1].bitcast(f32), 0.0)
            nc.gpsimd.memset(x3[:, 1:HP - 1, WP - 1:WP].bitcast(f32), 0.0)
            nc.gpsimd.tensor_copy(
                x3[:, 1:HP - 1, 1:WP - 1],
                xtmp.rearrange("p (h w) -> p h w", h=H))
            xqs.append(xq)

            # routing MLP -> rwb [128, 4] (rw broadcast to all partitions)
            gs = workp.tile([128, 1], f32, tag="gs", name=f"gs{b}")
            nc.vector.reduce_sum(gs, xtmp, axis=mybir.AxisListType.X)
            p1 = tinyps.tile([2, 1], f32, tag="tps", name=f"p1_{b}")
            nc.tensor.matmul(p1, C('r1wT', 128, 2), gs, start=True, stop=True)
            h1 = workp.tile([2, 1], f32, tag="h1", name=f"h1_{b}")
            nc.scalar.activation(h1, p1, AF.Relu, bias=C('r1b', 2, 1),
                                 scale=1.0 / HW)
            p2 = tinyps.tile([4, 1], f32, tag="tps", name=f"p2_{b}")
            nc.tensor.matmul(p2, C('r2wT', 2, 4), h1, start=True, stop=True)
            h2 = workp.tile([4, 1], f32, tag="h2", name=f"h2_{b}")
            nc.scalar.activation(h2, p2, AF.Relu, bias=C('r2b', 4, 1))
            p3 = tinyps.tile([4, 1], f32, tag="tps", name=f"p3_{b}")
            nc.tensor.matmul(p3, C('r3wT', 4, 4), h2, start=True, stop=True)
            e4 = workp.tile([4, 1], f32, tag="e4", name=f"e4_{b}")
            nc.scalar.activation(e4, p3, AF.Exp, bias=C('r3b', 4, 1))
            pse = tinyps.tile([1, 1], f32, tag="tps", name=f"pse{b}")
            nc.tensor.matmul(pse, C('ones49', SA_K, 1)[0:4, :], e4,
                             start=True, stop=True)
            se = workp.tile([1, 1], f32, tag="se", name=f"se{b}")
            nc.scalar.copy(se, pse)
            rcp = workp.tile([1, 1], f32, tag="rcp", name=f"rcp{b}")
            nc.vector.reciprocal(rcp, se)
            psT = tinyps.tile([1, 4], f32, tag="tps", name=f"psT{b}")
            nc.tensor.matmul(psT, e4, C('I4', 4, 4), start=True, stop=True)
            rwT = workp.tile([1, 4], f32, tag="rwT", name=f"rwT{b}")
            nc.vector.tensor_scalar_mul(rwT, psT, rcp)
            psB = tinyps.tile([128, 4], f32, tag="tps", name=f"psB{b}")
            nc.tensor.matmul(psB, C('ones_r', 1, 128), rwT, start=True, stop=True)
            rwb = workp.tile([128, 4], f32, tag="rwb", name=f"rwb{b}")
            nc.scalar.copy(rwb, psB)
            rwbs.append(rwb)

        # ---------------- stream experts, build W_eff per sample
        weffs = [weffp.tile([128, NOFF * COUT], f32r, tag="weff", name=f"weff{b}")
                 for b in range(BPC)]
        for e in range(E):
            ewt = ewp.tile([128, NOFF * COUT], f32, tag="ewt", name=f"ewt{e}")
            nc.sync.dma_start(ewt, ew_d[:, e * NOFF * COUT:(e + 1) * NOFF * COUT])
            for b in range(BPC):
                if e == 0:
                    nc.vector.tensor_scalar_mul(weffs[b], ewt, rwbs[b][:, 0:1])
                else:
                    nc.vector.scalar_tensor_tensor(
                        weffs[b], ewt, rwbs[b][:, e:e + 1], weffs[b],
                        alu.mult, alu.add)

        # ---------------- per-sample state
        obs, ob3s, pcolss = [], [], []
        tsums, sqcolss, cas = [], [], []
        for b in range(BPC):
            ob = outp.tile([128, 2 * HW], f32r, tag="ob", name=f"ob{b}")
            obs.append(ob)
            ob3s.append(ob.rearrange("p (h x) -> p h x", h=2))
            pc = workp.tile([128, 16], f32, tag="pcols", name=f"pcols{b}")
            pcolss.append(pc)
            tsums.append(workp.tile([128, 16], f32, tag="tsum", name=f"tsum{b}"))
            sqcolss.append(workp.tile([128, 16], f32, tag="sqcols",
                                      name=f"sqcols{b}"))
            cas.append(workp.tile([128, 2], f32, tag="ca", name=f"ca{b}"))

        def emit_conv_chunk(b, c):
            x3 = xqs[b].rearrange("p (h w) -> p h w", h=HP)
            for h in range(2):
                ps = cvps.tile([128, CH], f32, tag="cv", name=f"cv{b}_{c}_{h}")
                for o, (dy, dx) in enumerate(OFFS3):
                    lhsT = weffs[b][:, o * COUT + h * 128:
                                    o * COUT + h * 128 + 128]
                    rhs = x3[:, c * RPC + dy:c * RPC + dy + RPC, dx:dx + W]
                    nc.tensor.matmul(ps, lhsT, rhs, start=(o == 0),
                                     stop=(o == NOFF - 1))
                nc.scalar.activation(
                    ob3s[b][:, h, c * CH:(c + 1) * CH], ps, AF.Copy,
                    accum_out=pcolss[b][:, h * NCH + c:h * NCH + c + 1])

        def emit_sa_front(b):
            """Q matmuls, padded-Q assembly, 49 shifted row DMAs."""
            qpad = bigp.tile([SA_K, QPS], f32r, tag="big", name=f"qpad{b}")
            q3 = qpad[:, 0:QH * QW].rearrange("p (r c) -> p r c", r=QH)
            nc.gpsimd.memset(qpad[:, QH * QW:QPS].bitcast(f32), 0.0)
            nc.gpsimd.memset(q3[:, 0:3, :].bitcast(f32), 0.0)
            nc.gpsimd.memset(q3[:, QH - 3:QH, :].bitcast(f32), 0.0)
            nc.gpsimd.memset(q3[:, 3:QH - 3, 0:3].bitcast(f32), 0.0)
            nc.gpsimd.memset(q3[:, 3:QH - 3, QW - 3:QW].bitcast(f32), 0.0)
            for c in range(NCH):
                qp = qps.tile([SA_K, CH], f32, tag="q", name=f"q{b}_{c}")
                for h in range(2):
                    nc.tensor.matmul(qp, wsar[:, h, :],
                                     ob3s[b][:, h, c * CH:(c + 1) * CH],
                                     start=(h == 0), stop=(h == 1))
                nc.scalar.copy(
                    q3[:, 3 + c * RPC:3 + (c + 1) * RPC, 3:3 + W],
                    qp.rearrange("p (r c) -> p r c", r=RPC))
            # contiguous shifted spans: rb[p, i] = qpad[p, (70*dy+dx) + i]
            rb = bigp.tile([SA_K, RSP], f32r, tag="big", name=f"rb{b}")
            for p, (dy, dx) in enumerate(OFFS7):
                eng = nc.sync if p % 2 == 0 else nc.scalar
                off = QW * dy + dx
                eng.dma_start(rb[p:p + 1, :], qpad[p:p + 1, off:off + RSP])
            return rb

        def emit_s_chunk(b, rb, c):
            """sa sigmoid, broadcast, t1 = out*sa in place, stats."""
            sp = sbps.tile([1, CH], f32, tag="sb", name=f"sp{b}_{c}")
            rb3 = rb.rearrange("p (r w) -> p r w", w=QW)
            nc.tensor.matmul(sp, ones49r,
                             rb3[:, c * RPC:(c + 1) * RPC, 0:W],
                             start=True, stop=True)
            sac = sap.tile([1, CH], f32r, tag="sas", name=f"sac{b}_{c}")
            nc.scalar.activation(sac, sp, AF.Sigmoid, bias=C('sab', 1, 1))
            bp = sbps.tile([128, CH], f32, tag="sb", name=f"bp{b}_{c}")
            nc.tensor.matmul(bp, onesrr, sac, start=True, stop=True)
            for h in range(2):
                sl = ob3s[b][:, h, c * CH:(c + 1) * CH]
                nc.vector.scalar_tensor_tensor(
                    sl, sl, 1.0, bp, alu.mult, alu.mult,
                    accum_out=tsums[b][:, h * NCH + c:h * NCH + c + 1])
                sqs = workp.tile([128, CH], f32, tag="sqs",
                                 name=f"sqs{b}_{c}_{h}")
                nc.scalar.activation(
                    sqs, sl, AF.Square,
                    accum_out=sqcolss[b][:, h * NCH + c:h * NCH + c + 1])

        def emit_ca(b):
            pg = workp.tile([128, 2], f32, tag="pg", name=f"pg{b}")
            for h in range(2):
                nc.vector.reduce_sum(pg[:, h:h + 1],
                                     pcolss[b][:, h * NCH:(h + 1) * NCH],
                                     axis=mybir.AxisListType.X)
            pca = tinyps.tile([64, 1], f32, tag="tps", name=f"pca{b}")
            for h in range(2):
                nc.tensor.matmul(pca, C('ca1a' if h == 0 else 'ca1bw', 128, 64),
                                 pg[:, h:h + 1], start=(h == 0), stop=(h == 1))
            hca = workp.tile([64, 1], f32, tag="hca", name=f"hca{b}")
            nc.scalar.activation(hca, pca, AF.Relu, bias=C('ca1b', 64, 1),
                                 scale=1.0 / HW)
            for h in range(2):
                pc2 = tinyps.tile([128, 1], f32, tag="tps", name=f"pc2{b}_{h}")
                nc.tensor.matmul(pc2,
                                 C('ca2wT', 64, 256)[:, h * 128:(h + 1) * 128],
                                 hca, start=True, stop=True)
                nc.scalar.activation(cas[b][:, h:h + 1], pc2, AF.Sigmoid,
                                     bias=C('ca2b', 128, 2)[:, h:h + 1])

        # ---------------- main schedule
        # sample0 conv + its Q/R front
        for c in range(NCH):
            emit_conv_chunk(0, c)
        rb0 = emit_sa_front(0)
        emit_ca(0)
        # sample1 conv with sample0's sa/t1/stats interleaved between chunks
        for c in range(NCH):
            emit_conv_chunk(1, c)
            emit_s_chunk(0, rb0, c)
        rb1 = emit_sa_front(1)
        emit_ca(1)
        for c in range(NCH):
            emit_s_chunk(1, rb1, c)

        # ---------------- local stats [128, 4] = (sum h0, sum h1, sq h0, sq h1)
        statL = workp.tile([128, 4], f32, tag="statL")
        tsum2 = [workp.tile([128, 2], f32, tag="tsum2", name=f"tsum2_{b}")
                 for b in range(BPC)]
        sqsum = [workp.tile([128, 2], f32, tag="sqsum", name=f"sqsum{b}")
                 for b in range(BPC)]
        casq = [workp.tile([128, 2], f32, tag="casq", name=f"casq{b}")
                for b in range(BPC)]
        tmp2 = [workp.tile([128, 2], f32, tag="tmp2", name=f"tmp2_{b}")
                for b in range(BPC)]
        for b in range(BPC):
            for h in range(2):
                nc.vector.reduce_sum(tsum2[b][:, h:h + 1],
                                     tsums[b][:, h * NCH:(h + 1) * NCH],
                                     axis=mybir.AxisListType.X)
                nc.vector.reduce_sum(sqsum[b][:, h:h + 1],
                                     sqcolss[b][:, h * NCH:(h + 1) * NCH],
                                     axis=mybir.AxisListType.X)
            nc.vector.tensor_mul(casq[b], cas[b], cas[b])
            if b == 0:
                nc.vector.tensor_mul(statL[:, 0:2], cas[b], tsum2[b])
                nc.vector.tensor_mul(statL[:, 2:4], casq[b], sqsum[b])
            else:
                nc.vector.tensor_mul(tmp2[b], cas[b], tsum2[b])
                nc.vector.tensor_add(statL[:, 0:2], statL[:, 0:2], tmp2[b])
                nc.vector.tensor_mul(tmp2[b], casq[b], sqsum[b])
                nc.vector.tensor_add(statL[:, 2:4], statL[:, 2:4], tmp2[b])

        # ---------------- AllReduce across 8 cores
        statG = workp.tile([128, 4], f32, tag="statG")
        if USE_COLLECTIVE:
            cin_t = dramp.tile([128, 4], f32, tag="cin")
            cout_t = dramp.tile([128, 4], f32, tag="cout")
            nc.sync.dma_start(cin_t, statL)
            nc.gpsimd.collective_compute(
                "AllReduce", alu.add,
                replica_groups=[list(range(NCORES))],
                ins=[cin_t.opt()], outs=[cout_t.opt()])
            nc.sync.dma_start(statG, cout_t)
        else:
            # bisection mode: local stats scaled by NCORES (approximate BN)
            nc.vector.tensor_scalar_mul(statG, statL, float(NCORES))

        # ---------------- finalize: y = relu(t1 * (ca*s) + (beta - mean*s))
        NTOT = float(B * HW)
        mean = workp.tile([128, 2], f32, tag="mean")
        nc.vector.tensor_scalar_mul(mean, statG[:, 0:2], 1.0 / NTOT)
        msq = workp.tile([128, 2], f32, tag="msq")
        nc.vector.tensor_mul(msq, mean, mean)
        var = workp.tile([128, 2], f32, tag="var")
        nc.vector.scalar_tensor_tensor(var, statG[:, 2:4], 1.0 / NTOT, msq,
                                       alu.mult, alu.subtract)
        stdv = workp.tile([128, 2], f32, tag="stdv")
        nc.scalar.activation(stdv, var, AF.Sqrt, bias=C('eps', 128, 1))
        rstd = workp.tile([128, 2], f32, tag="rstd")
        nc.vector.reciprocal(rstd, stdv)
        sg = workp.tile([128, 2], f32, tag="sg")
        nc.vector.tensor_mul(sg, C('gamma', 128, 2), rstd)
        ms = workp.tile([128, 2], f32, tag="ms")
        nc.vector.tensor_mul(ms, mean, sg)
        tg = workp.tile([128, 2], f32, tag="tg")
        nc.vector.tensor_sub(tg, C('beta', 128, 2), ms)

        for b in range(BPC):
            casg = workp.tile([128, 2], f32, tag="casg", name=f"casg{b}")
            nc.vector.tensor_mul(casg, cas[b], sg)
            for h in range(2):
                nc.scalar.activation(ob3s[b][:, h, :], ob3s[b][:, h, :],
                                     AF.Relu, scale=casg[:, h:h + 1],
                                     bias=tg[:, h:h + 1])
                nc.sync.dma_start(y_d[b, h * 128:(h + 1) * 128].opt(),
                                  ob3s[b][:, h, :].bitcast(f32))


# ---------------------------------------------------------------- driver

_CACHE = {}


def _get_nc(ncst, cols_key, cols):
    key = (ncst, cols_key, USE_COLLECTIVE)
    if key not in _CACHE:
        nc = bacc.Bacc("TRN2", target_bir_lowering=False, debug=False,
                       num_devices=NCORES)
        _CACHE[key] = _emit(nc, cols, ncst)
    return _CACHE[key]


def _run(inputs, trace=False):
    cst, cols = _build_consts(inputs)
    ncst = cst.shape[1]
    nc = _get_nc(ncst, tuple(sorted(cols.items())), cols)

    x = np.ascontiguousarray(np.asarray(inputs['x'], np.float32))
    ew = np.ascontiguousarray(
        np.asarray(inputs['expert_w'], np.float32)
        .transpose(2, 0, 3, 4, 1).reshape(CIN, E * NOFF * COUT))

    in_maps = []
    for c in range(NCORES):
        in_maps.append({
            "x": np.ascontiguousarray(x[c * BPC:(c + 1) * BPC]),
            "ew": ew,
            "cst": cst,
        })
    br = bass_utils.run_bass_kernel_spmd(
        nc, in_maps, core_ids=list(range(NCORES)), trace=trace)
    y = np.concatenate([r["y"] for r in br.results], axis=0)
    return y.astype(np.float32, copy=False), br


def kernel(**inputs):
    y, _ = _run(inputs, trace=False)
    return y
